# revision 24
# baseline (speedup 1.0000x reference)
"""GCN encoder (3x GCNConv: 128->64 relu, 64->{32 mu, 32 logstd}) on 8 trn2
NeuronCores via Bass.

Math: with A_hat = D^-1/2 (A + I) D^-1/2 (self-loops included),
  GCNConv(x, W, b) = A_hat @ (x @ W) + b = dinv * segsum((x@W * dinv)[src]) + b
The two output convs share the aggregation: agg(h) @ [W_mu|W_ls] + [b_mu|b_ls].

Distribution: destination nodes sharded 8 ways (12500/core). Table rows are in
natural node order (row = local node index). Source windows are COLUMN BANDS
replicated across all 8 shards; two bands form a PAIR that is exchanged with a
single AllGather (strided output AP places each rank's rows into both band
regions), so the exchange is 2 x 1.6MB instead of 4 x 0.8MB and still overlaps
the gather pipeline.

Each core:
  stage A:  u = x_shard @ W1 (PE), row-scale by dinv -> pair tensors tA/tB;
            AllGather pair fires as soon as its columns complete.
  level 1:  per band (<=25601 table rows, int16 idx limit), dma_gather message
            rows in degree-class round order, segment-sum on DVE (one
            tensor_reduce per equal-degree run, L==1 included), partials -> P
  level 2:  per col-chunk: 4 small gathers align the per-band partials by dst
            + self-loop rows; fused post-ops; h pairs -> AllGather again.
  layer 2:  same index structure re-used on the h table; final PE matmul
            with [W_mu|W_ls]; output written feature-major, unsharded on host.
"""
import os
import numpy as np

N = 100_000
E = 1_600_000
IN, HID, OUT = 128, 64, 32
NC = 8
SH = N // NC            # 12500 dst nodes per core
LANES = 128
COLS = 98               # ceil(SH/128)
GRID = LANES * COLS     # 12544 (pos grid incl 44 junk)
NW = 4                  # src bands (windows)
BCOL = [(0, 25), (25, 50), (50, 74), (74, 98)]   # band col ranges
LB = [c0 * LANES for c0, _ in BCOL]              # band row starts (local)
BROWS = [(c1 - c0) * LANES for c0, c1 in BCOL]   # rows per band per shard
ZR = [8 * b for b in BROWS]                      # zero-row rel idx per band
PAIRC0 = [0, 50]                                 # first col of each pair
PROWS = [2 * (8 * BROWS[0] + 1), 2 * (8 * BROWS[2] + 1)]  # pair tensor rows
CALL_COLS = 64          # gather call capacity (8192 idxs)


# ---------------------------------------------------------------- structure
def build_structure(edge_index):
    src = np.asarray(edge_index[0], dtype=np.int64)
    dst = np.asarray(edge_index[1], dtype=np.int64)
    deg = (np.bincount(dst, minlength=N) + 1.0).astype(np.float32)
    dinv = (1.0 / np.sqrt(deg)).astype(np.float32)

    spos = src % SH
    cs = src // SH
    win = np.digitize(spos, LB[1:])              # 0..3
    rel = (cs * np.asarray(BROWS)[win] + (spos - np.asarray(LB)[win])).astype(
        np.int64)
    core = dst // SH
    ld = dst % SH

    # per (core, window) destination counts
    cnt = np.zeros((NC, NW, SH), np.int64)
    np.add.at(cnt, (core, win, ld), 1)

    # per (c,k): dst order by count desc (stable), rank of each dst
    order = np.argsort(-cnt + 0, axis=2, kind="stable")  # [NC, NW, SH]
    rank = np.empty_like(order)
    ar = np.arange(SH)
    for c in range(NC):
        for k in range(NW):
            rank[c, k, order[c, k]] = ar
    csort = np.take_along_axis(cnt, order, axis=2)       # counts sorted desc

    # common rounds: nr[k], L[k][r]
    nr = np.zeros(NW, np.int64)
    Ls = []
    for k in range(NW):
        npos = (cnt[:, k] > 0).sum(axis=1)               # [NC]
        nr[k] = int(-(-npos.max() // LANES))
        Lkr = np.zeros(nr[k], np.int64)
        for c in range(NC):
            m = int(-(-npos[c] // LANES))
            Lkr[:m] = np.maximum(Lkr[:m], csort[c, k, : m * LANES : LANES])
        assert (Lkr >= 1).all() and (Lkr <= CALL_COLS).all()
        Ls.append(Lkr)

    # pack rounds into gather calls (per window), runs of equal L per call
    calls = []          # (k, col0_in_stream, cols, r0, [(L, nRounds)...])
    slotbase = []       # per window: slot-col offset of each round
    for k in range(NW):
        sb = np.concatenate([[0], np.cumsum(Ls[k])])
        slotbase.append(sb)
        r = 0
        while r < nr[k]:
            r0, c0 = r, sb[r]
            while r < nr[k] and sb[r + 1] - c0 <= CALL_COLS:
                r += 1
            runs = []
            for rr in range(r0, r):
                L = int(Ls[k][rr])
                if runs and runs[-1][0] == L:
                    runs[-1][1] += 1
                else:
                    runs.append([L, 1])
            calls.append((k, int(c0), int(sb[r] - c0), r0, runs))
    total_cols = sum(c[2] for c in calls)

    # level-2 col-chunks: 2 per band, aligned to band boundaries
    chunks = []         # (band, c0, c1)
    for k, (c0, c1) in enumerate(BCOL):
        h = (c1 - c0) // 2
        chunks.append((k, c0, c0 + h))
        chunks.append((k, c0 + h, c1))

    # per-core data arrays
    per_core = []
    for c in range(NC):
        m = core == c
        ex_ld, ex_win, ex_rel = ld[m], win[m], rel[m]
        ix1 = np.zeros((128, total_cols * 8), np.int16)
        i2full = np.empty((NW, GRID), np.int64)
        cw = 0
        for k in range(NW):
            sb = slotbase[k]
            ncols = int(sb[nr[k]])
            flat = np.full(ncols * LANES, ZR[k], np.int64)
            mk = ex_win == k
            eld, erel = ex_ld[mk], ex_rel[mk]
            # seq number of each edge within its dst
            o = np.argsort(eld, kind="stable")
            eld_s, erel_s = eld[o], erel[o]
            firsts = np.searchsorted(eld_s, eld_s)
            seq = np.arange(len(eld_s)) - firsts
            q = rank[c, k, eld_s]
            col = sb[q // LANES] + seq
            lane = q % LANES
            flat[col * LANES + lane] = erel_s
            # slice into this window's calls, wrap each
            for (kk, c0, cols, _r0, _runs) in calls:
                if kk != k:
                    continue
                seg = flat[c0 * LANES:(c0 + cols) * LANES]
                w = seg.reshape(-1, 16).T.astype(np.int16)
                ix1[:, cw:cw + cols * 8] = np.tile(w, (8, 1))
                cw += cols * 8
            # level-2 idx for window k
            i2 = np.full(GRID, LANES * nr[k], np.int64)
            hasdst = cnt[c, k] > 0
            qq = rank[c, k, ar[hasdst]]
            i2[ar[hasdst]] = (qq % LANES) * nr[k] + qq // LANES
            i2full[k] = i2
        assert cw == total_cols * 8
        stream = []
        for ci, (kb, cc0, cc1) in enumerate(chunks):
            for k in range(NW):
                stream.append(i2full[k][cc0 * 128:cc1 * 128])
        flat2 = np.concatenate(stream)
        w2 = flat2.reshape(-1, 16).T.astype(np.int16)
        ix2 = np.tile(w2, (8, 1))

        nodes = np.arange(SH) + c * SH
        dvgrid = np.ones((LANES, COLS), np.float32)
        dvgrid[np.arange(SH) % LANES, np.arange(SH) // LANES] = dinv[nodes]
        per_core.append(dict(ix1=ix1, ix2=ix2, dvgrid=dvgrid))

    ix2off = {}
    pos = 0
    for ci, (kb, cc0, cc1) in enumerate(chunks):
        for k in range(NW):
            ix2off[(ci, k)] = pos // 16
            pos += (cc1 - cc0) * 128
    return dict(dinv=dinv, calls=calls, nr=nr, chunks=chunks, ix2off=ix2off,
                total_cols=total_cols, per_core=per_core)


def make_inputs(st, x, W1, b1, W_mu, b_mu, W_ls, b_ls):
    """Per-core in_maps for the bass kernel."""
    W2 = np.concatenate([W_mu, W_ls], axis=1).astype(np.float32)     # [64, 64]
    b2 = np.concatenate([b_mu, b_ls]).astype(np.float32)[:, None]    # [64, 1]
    in_maps = []
    for c in range(NC):
        pc = st["per_core"][c]
        dvg = pc["dvgrid"]                                           # [128, 98]
        A = (dvg * dvg)[:, :, None] * np.ones((1, 1, HID), np.float32)
        B = dvg[:, :, None] * b1[None, None, :].astype(np.float32)
        D = dvg[:, :, None] * np.ones((1, 1, HID), np.float32)
        xT = np.ascontiguousarray(x[c * SH:(c + 1) * SH].T)          # [128, 12500]
        in_maps.append({
            "xT": xT.astype(np.float32),
            "W1": np.asarray(W1, np.float32),
            "W2": W2,
            "b2": b2,
            "dinvc": dvg,
            "Aexp": np.ascontiguousarray(A.reshape(LANES, COLS * HID)),
            "Bexp": np.ascontiguousarray(B.reshape(LANES, COLS * HID)),
            "Dexp": np.ascontiguousarray(D.reshape(LANES, COLS * HID)),
            "ix1": pc["ix1"],
            "ix2": pc["ix2"],
        })
    return in_maps


# ---------------------------------------------------------------- program
def build_program(st, do_compile=True):
    import concourse.bacc as bacc
    import concourse.mybir as mybir
    import concourse.tile as tile
    from concourse.masks import make_identity

    f32, i16 = mybir.dt.float32, mybir.dt.int16
    AF = mybir.ActivationFunctionType
    calls, nr, chunks = st["calls"], st["nr"], st["chunks"]
    total_cols = st["total_cols"]

    nc = bacc.Bacc("TRN2", target_bir_lowering=False, debug=False,
                   num_devices=NC, num_swdge_queues=4)
    xT = nc.dram_tensor("xT", [IN, SH], f32, kind="ExternalInput").ap()
    W1t = nc.dram_tensor("W1", [IN, HID], f32, kind="ExternalInput").ap()
    W2t = nc.dram_tensor("W2", [HID, HID], f32, kind="ExternalInput").ap()
    b2t = nc.dram_tensor("b2", [HID, 1], f32, kind="ExternalInput").ap()
    dinvc = nc.dram_tensor("dinvc", [LANES, COLS], f32, kind="ExternalInput").ap()
    Aexp = nc.dram_tensor("Aexp", [LANES, COLS * HID], f32, kind="ExternalInput").ap()
    Bexp = nc.dram_tensor("Bexp", [LANES, COLS * HID], f32, kind="ExternalInput").ap()
    Dexp = nc.dram_tensor("Dexp", [LANES, COLS * HID], f32, kind="ExternalInput").ap()
    ix1t = nc.dram_tensor("ix1", [128, total_cols * 8], i16, kind="ExternalInput").ap()
    ix2t = nc.dram_tensor("ix2", [128, NW * GRID // 16], i16, kind="ExternalInput").ap()
    outT = nc.dram_tensor("outT", [HID, GRID], f32, kind="ExternalOutput").ap()

    # per-band tensors: local shard bands, gathered tables, h bands
    tb = [nc.dram_tensor(f"tb{k}", [BROWS[k], HID], f32, kind="Internal")
          for k in range(NW)]
    hb = [nc.dram_tensor(f"hb{k}", [BROWS[k], HID], f32, kind="Internal")
          for k in range(NW)]
    t1 = [nc.dram_tensor(f"t1b{k}", [8 * BROWS[k] + 1, HID], f32,
                         kind="Internal", addr_space="Shared")
          for k in range(NW)]
    t2 = [nc.dram_tensor(f"t2b{k}", [8 * BROWS[k] + 1, HID], f32,
                         kind="Internal", addr_space="Shared")
          for k in range(NW)]
    P = [[nc.dram_tensor(f"P{l}_{k}", [LANES * int(nr[k]) + 1, HID], f32,
                         kind="Internal")
          for k in range(NW)] for l in range(2)]

    def band_in(tp, k):
        return tp[k].ap()[:, :]

    def ag_band(loc, tp, k):
        nc.gpsimd.collective_compute(
            "AllGather", mybir.AluOpType.bypass,
            replica_groups=[list(range(NC))],
            ins=[loc[k].ap().opt()],
            outs=[tp[k].ap()[:8 * BROWS[k], :].opt()])

    def pm(loc_list, k, csl):
        """[p, c, f] view of band tensor k for GLOBAL grid col slice."""
        c0b = BCOL[k][0]
        v = loc_list[k].ap().rearrange("(c p) f -> p c f", p=LANES)
        return v[:, csl.start - c0b:csl.stop - c0b, :]

    def prowview(l, k, r0, n):
        v = P[l][k].ap()[:LANES * int(nr[k]), :].rearrange(
            "(p r) f -> p r f", p=LANES)
        return v[:, r0:r0 + n, :]

    qrot = [0]

    def nextq():
        q = qrot[0] % 4
        qrot[0] += 1
        return q

    nidx_regs = {}

    def nidx_reg(v):
        # per-call int num_idxs_reg emits a reg write into ONE shared
        # register; the WAR on it serializes calls behind the previous
        # drain. Hoist one read-only register per distinct value.
        if v not in nidx_regs:
            nidx_regs[v] = nc.gpsimd.to_reg(v)
        return nidx_regs[v]

    def gather(out_ap, in_ap, idxs_ap, num_idxs):
        nc.gpsimd.dma_gather(
            out_ap=out_ap, in_ap=in_ap, idxs_ap=idxs_ap,
            num_idxs=num_idxs, num_idxs_reg=nidx_reg(num_idxs),
            elem_size=HID, single_packet=False, queue_num=nextq())

    with tile.TileContext(nc) as tc:
        with (
            tc.tile_pool(name="const", bufs=1) as cst,
            tc.tile_pool(name="ix", bufs=1) as ixp,
            tc.tile_pool(name="mm", bufs=1) as mmp,
            tc.tile_pool(name="ga", bufs=1) as gap,
            tc.tile_pool(name="st", bufs=1) as stp,
            tc.tile_pool(name="l2", bufs=1) as l2p,
            tc.tile_pool(name="ps", bufs=2, space="PSUM") as psp,
        ):
            ident = cst.tile([128, 128], f32)
            make_identity(nc, ident[:])
            w1 = cst.tile([IN, HID], f32)
            nc.sync.dma_start(out=w1[:], in_=W1t[:, :])
            w2 = cst.tile([HID, HID], f32)
            nc.sync.dma_start(out=w2[:], in_=W2t[:, :])
            b2 = cst.tile([HID, 1], f32)
            nc.sync.dma_start(out=b2[:], in_=b2t[:, :])
            dvc = cst.tile([LANES, COLS], f32)
            nc.sync.dma_start(out=dvc[:], in_=dinvc[:, :])
            zt = cst.tile([128, HID], f32)
            nc.gpsimd.memset(zt[:], 0.0)
            ix1 = ixp.tile([128, total_cols * 8], i16)
            nc.sync.dma_start(out=ix1[:], in_=ix1t[:, :])
            ix2 = ixp.tile([128, NW * GRID // 16], i16)
            nc.sync.dma_start(out=ix2[:], in_=ix2t[:, :])

            # zero rows: table-band zero rows, P region tails
            for k in range(NW):
                nc.sync.dma_start(out=t1[k].ap()[ZR[k]:ZR[k] + 1, :],
                                  in_=zt[:1, :])
                nc.sync.dma_start(out=t2[k].ap()[ZR[k]:ZR[k] + 1, :],
                                  in_=zt[:1, :])
                zr = LANES * int(nr[k])
                for l in range(2):
                    nc.sync.dma_start(out=P[l][k].ap()[zr:zr + 1, :],
                                      in_=zt[:1, :])

            # ---------------- stage A: u = xT.T @ W1, scale, store pair rows;
            # AllGather each pair as soon as its columns are written.
            ag_after = {6: 0, 12: 1, 18: 2, 24: 3}  # chunk j -> band done
            nchunk = -(-SH // 512)
            for j in range(nchunk):
                n = min(512, SH - j * 512)
                xt = mmp.tile([IN, 512], f32, tag="xt", bufs=3)
                nc.sync.dma_start(out=xt[:, :n], in_=xT[:, j * 512:j * 512 + n])
                pu = psp.tile([HID, 512], f32, tag="pu", bufs=2)
                nc.tensor.matmul(pu[:, :n], lhsT=w1[:], rhs=xt[:, :n],
                                 start=True, stop=True)
                su = mmp.tile([HID, 512], f32, tag="su", bufs=3)
                nc.scalar.activation(su[:, :n], pu[:, :n], AF.Copy)
                for i in range(-(-n // 128)):
                    b = min(128, n - i * 128)
                    col = 4 * j + i
                    k = next(kk for kk in range(NW)
                             if BCOL[kk][0] <= col < BCOL[kk][1])
                    pt = psp.tile([128, HID], f32, tag="pt", bufs=2)
                    nc.tensor.transpose(out=pt[:b, :],
                                        in_=su[:, i * 128:i * 128 + b],
                                        identity=ident[:HID, :HID])
                    ut = mmp.tile([128, HID], f32, tag="ut", bufs=4)
                    nc.scalar.activation(ut[:b, :], pt[:b, :], AF.Copy,
                                         scale=dvc[:b, col:col + 1])
                    r0 = (col - BCOL[k][0]) * LANES
                    nc.sync.dma_start(out=tb[k].ap()[r0:r0 + b, :],
                                      in_=ut[:b, :])
                if j in ag_after:
                    ag_band(tb, t1, ag_after[j])

            # ---------------- two gather layers
            for layer, (tp, selfsrc) in enumerate([(t1, tb), (t2, hb)]):
                # level 1: per-call gather + segment reduce -> P
                cw = 0
                for (k, c0, cols, r0, runs) in calls:
                    gt = gap.tile([128, CALL_COLS, HID], f32, tag="gt", bufs=4)
                    gather(gt[:, :cols, :], band_in(tp, k),
                           ix1[:, cw:cw + cols * 8], cols * 128)
                    cw += cols * 8
                    # all runs go through DVE (L==1 is a converting copy):
                    # keeps gt readers vector-only so gather-buffer reuse
                    # never waits on the shared HW-DMA lane semaphores.
                    nstage = sum(nR for (L, nR) in runs)
                    stt = stp.tile([128, CALL_COLS // 2, HID], f32,
                                   tag="stt", bufs=4, name="stt")
                    soff, coff = 0, 0
                    for (L, nR) in runs:
                        nc.vector.tensor_reduce(
                            out=stt[:, soff:soff + nR, :],
                            in_=gt[:, coff:coff + nR * L, :].rearrange(
                                "p (r l) f -> p r f l", l=L),
                            axis=mybir.AxisListType.X,
                            op=mybir.AluOpType.add)
                        soff += nR
                        coff += nR * L
                    nc.sync.dma_start(out=prowview(layer, k, r0, nstage),
                                      in_=stt[:, :nstage, :])

                # level 2: align partials + self, post-ops
                for ci, (kb, cc0, cc1) in enumerate(chunks):
                    ncc = cc1 - cc0
                    csl = slice(cc0, cc1)
                    parts = []
                    for k in range(NW):
                        g2 = l2p.tile([128, 13, HID], f32, tag="g2", bufs=5)
                        w0 = st["ix2off"][(ci, k)]
                        gather(g2[:, :ncc, :], P[layer][k].ap()[:, :],
                               ix2[:, w0:w0 + ncc * 8], ncc * 128)
                        parts.append(g2)
                    sf = l2p.tile([128, 13, HID], f32, tag="sf", bufs=3)
                    nc.sync.dma_start(out=sf[:, :ncc, :],
                                      in_=pm(selfsrc, kb, csl))
                    s = l2p.tile([128, 13, HID], f32, tag="s", bufs=2)
                    nc.vector.tensor_add(s[:, :ncc, :], parts[0][:, :ncc, :],
                                         parts[1][:, :ncc, :])
                    nc.vector.tensor_add(s[:, :ncc, :], s[:, :ncc, :],
                                         parts[2][:, :ncc, :])
                    nc.vector.tensor_add(s[:, :ncc, :], s[:, :ncc, :],
                                         parts[3][:, :ncc, :])
                    nc.vector.tensor_add(s[:, :ncc, :], s[:, :ncc, :],
                                         sf[:, :ncc, :])
                    sfl = s[:, :ncc, :].rearrange("p c f -> p (c f)")
                    if layer == 0:
                        a_t = l2p.tile([128, 13 * HID], f32, tag="ab", bufs=2)
                        nc.sync.dma_start(
                            out=a_t[:, :ncc * HID],
                            in_=Aexp[:, cc0 * HID:cc1 * HID])
                        b_t = l2p.tile([128, 13 * HID], f32, tag="bb", bufs=2)
                        nc.sync.dma_start(
                            out=b_t[:, :ncc * HID],
                            in_=Bexp[:, cc0 * HID:cc1 * HID])
                        nc.vector.tensor_mul(sfl, sfl, a_t[:, :ncc * HID])
                        nc.vector.tensor_add(sfl, sfl, b_t[:, :ncc * HID])
                        h = l2p.tile([128, 13 * HID], f32, tag="h", bufs=2)
                        nc.scalar.activation(h[:, :ncc * HID], sfl, AF.Relu)
                        nc.sync.dma_start(
                            out=pm(hb, kb, csl),
                            in_=h[:, :ncc * HID].rearrange(
                                "p (c f) -> p c f", f=HID))
                        if cc1 == BCOL[kb][1]:
                            ag_band(hb, t2, kb)
                    else:
                        d_t = l2p.tile([128, 13 * HID], f32, tag="ab", bufs=2)
                        nc.sync.dma_start(
                            out=d_t[:, :ncc * HID],
                            in_=Dexp[:, cc0 * HID:cc1 * HID])
                        nc.vector.tensor_mul(sfl, sfl, d_t[:, :ncc * HID])
                        # final: per 4-col block transpose + matmul + bias
                        nb = -(-ncc // 4)
                        for bI in range(nb):
                            bc = min(4, ncc - bI * 4)
                            pvt = psp.tile([HID, 512], f32, tag="pvt", bufs=2)
                            for i in range(bc):
                                nc.tensor.transpose(
                                    out=pvt[:, i * 128:(i + 1) * 128],
                                    in_=s[:, bI * 4 + i, :],
                                    identity=ident[:, :])
                            vt = mmp.tile([HID, 512], f32, tag="vt", bufs=3)
                            nc.scalar.activation(vt[:, :bc * 128],
                                                 pvt[:, :bc * 128], AF.Copy)
                            po = psp.tile([HID, 512], f32, tag="po", bufs=2)
                            nc.tensor.matmul(po[:, :bc * 128], lhsT=w2[:],
                                             rhs=vt[:, :bc * 128],
                                             start=True, stop=True)
                            ot = mmp.tile([HID, 512], f32, tag="ot", bufs=3)
                            nc.vector.tensor_scalar_add(ot[:, :bc * 128],
                                                        po[:, :bc * 128],
                                                        b2[:, 0:1])
                            c0g = (cc0 + bI * 4) * 128
                            nc.sync.dma_start(
                                out=outT[:, c0g:c0g + bc * 128],
                                in_=ot[:, :bc * 128])

    if do_compile:
        nc.compile()
    return nc


# ---------------------------------------------------------------- kernel
def kernel(x, edge_index, W1, b1, W_mu, b_mu, W_ls, b_ls):
    x = np.asarray(x, np.float32)
    edge_index = np.asarray(edge_index)
    st = build_structure(edge_index)
    in_maps = make_inputs(st, x, np.asarray(W1), np.asarray(b1),
                          np.asarray(W_mu), np.asarray(b_mu),
                          np.asarray(W_ls), np.asarray(b_ls))
    nc = build_program(st)
    from concourse import bass_utils
    trace = bool(int(os.environ.get("GCN_TRACE", "0")))
    if trace:
        try:
            import axon_profile_shim
            axon_profile_shim.install()
        except Exception:
            trace = False
    res = bass_utils.run_bass_kernel_spmd(
        nc, in_maps, core_ids=list(range(NC)), trace=trace)
    if trace:
        kernel.last_exec_time_ns = res.exec_time_ns
    mu = np.empty((N, OUT), np.float32)
    ls = np.empty((N, OUT), np.float32)
    for c in range(NC):
        o = res.results[c]["outT"]                     # [64, GRID]
        sel = o[:, :SH].T                              # [12500, 64]
        mu[c * SH:(c + 1) * SH] = sel[:, :OUT]
        ls[c * SH:(c + 1) * SH] = sel[:, OUT:]
    return mu, ls


# revision 25
# speedup vs baseline: 1.2815x; 1.2815x over previous
"""GCN encoder (3x GCNConv: 128->64 relu, 64->{32 mu, 32 logstd}) on 8 trn2
NeuronCores via Bass.

Math: with A_hat = D^-1/2 (A + I) D^-1/2 (self-loops included),
  GCNConv(x, W, b) = A_hat @ (x @ W) + b = dinv * segsum((x@W * dinv)[src]) + b
The two output convs share the aggregation: agg(h) @ [W_mu|W_ls] + [b_mu|b_ls].

Distribution: destination nodes sharded 8 ways (12500/core). Table rows are in
natural node order (row = local node index). Source windows are COLUMN BANDS
replicated across all 8 shards; two bands form a PAIR that is exchanged with a
single AllGather (strided output AP places each rank's rows into both band
regions), so the exchange is 2 x 1.6MB instead of 4 x 0.8MB and still overlaps
the gather pipeline.

Each core:
  stage A:  u = x_shard @ W1 (PE), row-scale by dinv -> pair tensors tA/tB;
            AllGather pair fires as soon as its columns complete.
  level 1:  per band (<=25601 table rows, int16 idx limit), dma_gather message
            rows in degree-class round order, segment-sum on DVE (one
            tensor_reduce per equal-degree run, L==1 included), partials -> P
  level 2:  per col-chunk: 4 small gathers align the per-band partials by dst
            + self-loop rows; fused post-ops; h pairs -> AllGather again.
  layer 2:  same index structure re-used on the h table; final PE matmul
            with [W_mu|W_ls]; output written feature-major, unsharded on host.
"""
import os
import numpy as np

N = 100_000
E = 1_600_000
IN, HID, OUT = 128, 64, 32
NC = 8
SH = N // NC            # 12500 dst nodes per core
LANES = 128
COLS = 98               # ceil(SH/128)
GRID = LANES * COLS     # 12544 (pos grid incl 44 junk)
NW = 4                  # src bands (windows)
BCOL = [(0, 25), (25, 50), (50, 74), (74, 98)]   # band col ranges
LB = [c0 * LANES for c0, _ in BCOL]              # band row starts (local)
BROWS = [(c1 - c0) * LANES for c0, c1 in BCOL]   # rows per band per shard
ZR = [8 * b for b in BROWS]                      # zero-row rel idx per band
PAIRC0 = [0, 50]                                 # first col of each pair
PROWS = [2 * (8 * BROWS[0] + 1), 2 * (8 * BROWS[2] + 1)]  # pair tensor rows
CALL_COLS = 32          # gather call capacity (4096 idxs)


# ---------------------------------------------------------------- structure
def build_structure(edge_index):
    src = np.asarray(edge_index[0], dtype=np.int64)
    dst = np.asarray(edge_index[1], dtype=np.int64)
    deg = (np.bincount(dst, minlength=N) + 1.0).astype(np.float32)
    dinv = (1.0 / np.sqrt(deg)).astype(np.float32)

    spos = src % SH
    cs = src // SH
    win = np.digitize(spos, LB[1:])              # 0..3
    rel = (cs * np.asarray(BROWS)[win] + (spos - np.asarray(LB)[win])).astype(
        np.int64)
    core = dst // SH
    ld = dst % SH

    # per (core, window) destination counts
    cnt = np.zeros((NC, NW, SH), np.int64)
    np.add.at(cnt, (core, win, ld), 1)

    # per (c,k): dst order by count desc (stable), rank of each dst
    order = np.argsort(-cnt + 0, axis=2, kind="stable")  # [NC, NW, SH]
    rank = np.empty_like(order)
    ar = np.arange(SH)
    for c in range(NC):
        for k in range(NW):
            rank[c, k, order[c, k]] = ar
    csort = np.take_along_axis(cnt, order, axis=2)       # counts sorted desc

    # common rounds: nr[k], L[k][r]
    nr = np.zeros(NW, np.int64)
    Ls = []
    for k in range(NW):
        npos = (cnt[:, k] > 0).sum(axis=1)               # [NC]
        nr[k] = int(-(-npos.max() // LANES))
        Lkr = np.zeros(nr[k], np.int64)
        for c in range(NC):
            m = int(-(-npos[c] // LANES))
            Lkr[:m] = np.maximum(Lkr[:m], csort[c, k, : m * LANES : LANES])
        assert (Lkr >= 1).all() and (Lkr <= CALL_COLS).all()
        Ls.append(Lkr)

    # pack rounds into gather calls (per window), runs of equal L per call
    calls = []          # (k, col0_in_stream, cols, r0, [(L, nRounds)...])
    slotbase = []       # per window: slot-col offset of each round
    for k in range(NW):
        sb = np.concatenate([[0], np.cumsum(Ls[k])])
        slotbase.append(sb)
        r = 0
        while r < nr[k]:
            r0, c0 = r, sb[r]
            while r < nr[k] and sb[r + 1] - c0 <= CALL_COLS:
                r += 1
            runs = []
            for rr in range(r0, r):
                L = int(Ls[k][rr])
                if runs and runs[-1][0] == L:
                    runs[-1][1] += 1
                else:
                    runs.append([L, 1])
            calls.append((k, int(c0), int(sb[r] - c0), r0, runs))
    total_cols = sum(c[2] for c in calls)

    # level-2 col-chunks: 2 per band, aligned to band boundaries
    chunks = []         # (band, c0, c1)
    for k, (c0, c1) in enumerate(BCOL):
        h = (c1 - c0) // 2
        chunks.append((k, c0, c0 + h))
        chunks.append((k, c0 + h, c1))

    # per-core data arrays
    per_core = []
    for c in range(NC):
        m = core == c
        ex_ld, ex_win, ex_rel = ld[m], win[m], rel[m]
        ix1 = np.zeros((128, total_cols * 8), np.int16)
        i2full = np.empty((NW, GRID), np.int64)
        cw = 0
        for k in range(NW):
            sb = slotbase[k]
            ncols = int(sb[nr[k]])
            flat = np.full(ncols * LANES, ZR[k], np.int64)
            mk = ex_win == k
            eld, erel = ex_ld[mk], ex_rel[mk]
            # seq number of each edge within its dst
            o = np.argsort(eld, kind="stable")
            eld_s, erel_s = eld[o], erel[o]
            firsts = np.searchsorted(eld_s, eld_s)
            seq = np.arange(len(eld_s)) - firsts
            q = rank[c, k, eld_s]
            col = sb[q // LANES] + seq
            lane = q % LANES
            flat[col * LANES + lane] = erel_s
            # slice into this window's calls, wrap each
            for (kk, c0, cols, _r0, _runs) in calls:
                if kk != k:
                    continue
                seg = flat[c0 * LANES:(c0 + cols) * LANES]
                w = seg.reshape(-1, 16).T.astype(np.int16)
                ix1[:, cw:cw + cols * 8] = np.tile(w, (8, 1))
                cw += cols * 8
            # level-2 idx for window k
            i2 = np.full(GRID, LANES * nr[k], np.int64)
            hasdst = cnt[c, k] > 0
            qq = rank[c, k, ar[hasdst]]
            i2[ar[hasdst]] = (qq % LANES) * nr[k] + qq // LANES
            i2full[k] = i2
        assert cw == total_cols * 8
        stream = []
        for ci, (kb, cc0, cc1) in enumerate(chunks):
            for k in range(NW):
                stream.append(i2full[k][cc0 * 128:cc1 * 128])
        flat2 = np.concatenate(stream)
        w2 = flat2.reshape(-1, 16).T.astype(np.int16)
        ix2 = np.tile(w2, (8, 1))

        nodes = np.arange(SH) + c * SH
        dvgrid = np.ones((LANES, COLS), np.float32)
        dvgrid[np.arange(SH) % LANES, np.arange(SH) // LANES] = dinv[nodes]
        per_core.append(dict(ix1=ix1, ix2=ix2, dvgrid=dvgrid))

    ix2off = {}
    pos = 0
    for ci, (kb, cc0, cc1) in enumerate(chunks):
        for k in range(NW):
            ix2off[(ci, k)] = pos // 16
            pos += (cc1 - cc0) * 128
    return dict(dinv=dinv, calls=calls, nr=nr, chunks=chunks, ix2off=ix2off,
                total_cols=total_cols, per_core=per_core)


def make_inputs(st, x, W1, b1, W_mu, b_mu, W_ls, b_ls):
    """Per-core in_maps for the bass kernel."""
    W2 = np.concatenate([W_mu, W_ls], axis=1).astype(np.float32)     # [64, 64]
    b2 = np.concatenate([b_mu, b_ls]).astype(np.float32)[:, None]    # [64, 1]
    in_maps = []
    for c in range(NC):
        pc = st["per_core"][c]
        dvg = pc["dvgrid"]                                           # [128, 98]
        A = (dvg * dvg)[:, :, None] * np.ones((1, 1, HID), np.float32)
        B = dvg[:, :, None] * b1[None, None, :].astype(np.float32)
        D = dvg[:, :, None] * np.ones((1, 1, HID), np.float32)
        xT = np.ascontiguousarray(x[c * SH:(c + 1) * SH].T)          # [128, 12500]
        in_maps.append({
            "xT": xT.astype(np.float32),
            "W1": np.asarray(W1, np.float32),
            "W2": W2,
            "b2": b2,
            "dinvc": dvg,
            "Aexp": np.ascontiguousarray(A.reshape(LANES, COLS * HID)),
            "Bexp": np.ascontiguousarray(B.reshape(LANES, COLS * HID)),
            "Dexp": np.ascontiguousarray(D.reshape(LANES, COLS * HID)),
            "ix1": pc["ix1"],
            "ix2": pc["ix2"],
        })
    return in_maps


# ---------------------------------------------------------------- program
def build_program(st, do_compile=True):
    import concourse.bacc as bacc
    import concourse.mybir as mybir
    import concourse.tile as tile
    from concourse.masks import make_identity

    f32, i16 = mybir.dt.float32, mybir.dt.int16
    AF = mybir.ActivationFunctionType
    calls, nr, chunks = st["calls"], st["nr"], st["chunks"]
    total_cols = st["total_cols"]

    nc = bacc.Bacc("TRN2", target_bir_lowering=False, debug=False,
                   num_devices=NC, num_swdge_queues=4)
    xT = nc.dram_tensor("xT", [IN, SH], f32, kind="ExternalInput").ap()
    W1t = nc.dram_tensor("W1", [IN, HID], f32, kind="ExternalInput").ap()
    W2t = nc.dram_tensor("W2", [HID, HID], f32, kind="ExternalInput").ap()
    b2t = nc.dram_tensor("b2", [HID, 1], f32, kind="ExternalInput").ap()
    dinvc = nc.dram_tensor("dinvc", [LANES, COLS], f32, kind="ExternalInput").ap()
    Aexp = nc.dram_tensor("Aexp", [LANES, COLS * HID], f32, kind="ExternalInput").ap()
    Bexp = nc.dram_tensor("Bexp", [LANES, COLS * HID], f32, kind="ExternalInput").ap()
    Dexp = nc.dram_tensor("Dexp", [LANES, COLS * HID], f32, kind="ExternalInput").ap()
    ix1t = nc.dram_tensor("ix1", [128, total_cols * 8], i16, kind="ExternalInput").ap()
    ix2t = nc.dram_tensor("ix2", [128, NW * GRID // 16], i16, kind="ExternalInput").ap()
    outT = nc.dram_tensor("outT", [HID, GRID], f32, kind="ExternalOutput").ap()

    # per-band tensors: local shard bands, gathered tables, h bands
    tb = [nc.dram_tensor(f"tb{k}", [BROWS[k], HID], f32, kind="Internal")
          for k in range(NW)]
    hb = [nc.dram_tensor(f"hb{k}", [BROWS[k], HID], f32, kind="Internal")
          for k in range(NW)]
    t1 = [nc.dram_tensor(f"t1b{k}", [8 * BROWS[k] + 1, HID], f32,
                         kind="Internal", addr_space="Shared")
          for k in range(NW)]
    t2 = [nc.dram_tensor(f"t2b{k}", [8 * BROWS[k] + 1, HID], f32,
                         kind="Internal", addr_space="Shared")
          for k in range(NW)]
    P = [[nc.dram_tensor(f"P{l}_{k}", [LANES * int(nr[k]) + 1, HID], f32,
                         kind="Internal")
          for k in range(NW)] for l in range(2)]

    def band_in(tp, k):
        return tp[k].ap()[:, :]

    def ag_band(loc, tp, k):
        nc.gpsimd.collective_compute(
            "AllGather", mybir.AluOpType.bypass,
            replica_groups=[list(range(NC))],
            ins=[loc[k].ap().opt()],
            outs=[tp[k].ap()[:8 * BROWS[k], :].opt()])

    def pm(loc_list, k, csl):
        """[p, c, f] view of band tensor k for GLOBAL grid col slice."""
        c0b = BCOL[k][0]
        v = loc_list[k].ap().rearrange("(c p) f -> p c f", p=LANES)
        return v[:, csl.start - c0b:csl.stop - c0b, :]

    def prowview(l, k, r0, n):
        v = P[l][k].ap()[:LANES * int(nr[k]), :].rearrange(
            "(p r) f -> p r f", p=LANES)
        return v[:, r0:r0 + n, :]

    qrot = [0]

    def nextq():
        q = qrot[0] % 4
        qrot[0] += 1
        return q

    nidx_regs = {}

    def nidx_reg(v):
        # per-call int num_idxs_reg emits a reg write into ONE shared
        # register; the WAR on it serializes calls behind the previous
        # drain. Hoist one read-only register per distinct value.
        if v not in nidx_regs:
            nidx_regs[v] = nc.gpsimd.to_reg(v)
        return nidx_regs[v]

    def gather(out_ap, in_ap, idxs_ap, num_idxs):
        nc.gpsimd.dma_gather(
            out_ap=out_ap, in_ap=in_ap, idxs_ap=idxs_ap,
            num_idxs=num_idxs, num_idxs_reg=nidx_reg(num_idxs),
            elem_size=HID, single_packet=False, queue_num=nextq())

    with tile.TileContext(nc) as tc:
        with (
            tc.tile_pool(name="const", bufs=1) as cst,
            tc.tile_pool(name="ix", bufs=1) as ixp,
            tc.tile_pool(name="mm", bufs=1) as mmp,
            tc.tile_pool(name="ga", bufs=1) as gap,
            tc.tile_pool(name="st", bufs=1) as stp,
            tc.tile_pool(name="l2", bufs=1) as l2p,
            tc.tile_pool(name="ps", bufs=2, space="PSUM") as psp,
        ):
            ident = cst.tile([128, 128], f32)
            make_identity(nc, ident[:])
            w1 = cst.tile([IN, HID], f32)
            nc.sync.dma_start(out=w1[:], in_=W1t[:, :])
            w2 = cst.tile([HID, HID], f32)
            nc.sync.dma_start(out=w2[:], in_=W2t[:, :])
            b2 = cst.tile([HID, 1], f32)
            nc.sync.dma_start(out=b2[:], in_=b2t[:, :])
            dvc = cst.tile([LANES, COLS], f32)
            nc.sync.dma_start(out=dvc[:], in_=dinvc[:, :])
            zt = cst.tile([128, HID], f32)
            nc.gpsimd.memset(zt[:], 0.0)
            ix1 = ixp.tile([128, total_cols * 8], i16)
            nc.sync.dma_start(out=ix1[:], in_=ix1t[:, :])
            ix2 = ixp.tile([128, NW * GRID // 16], i16)
            nc.sync.dma_start(out=ix2[:], in_=ix2t[:, :])

            # zero rows: table-band zero rows, P region tails
            for k in range(NW):
                nc.sync.dma_start(out=t1[k].ap()[ZR[k]:ZR[k] + 1, :],
                                  in_=zt[:1, :])
                nc.sync.dma_start(out=t2[k].ap()[ZR[k]:ZR[k] + 1, :],
                                  in_=zt[:1, :])
                zr = LANES * int(nr[k])
                for l in range(2):
                    nc.sync.dma_start(out=P[l][k].ap()[zr:zr + 1, :],
                                      in_=zt[:1, :])

            # ---------------- stage A: u = xT.T @ W1, scale, store pair rows;
            # AllGather each pair as soon as its columns are written.
            ag_after = {6: 0, 12: 1, 18: 2, 24: 3}  # chunk j -> band done
            nchunk = -(-SH // 512)
            for j in range(nchunk):
                n = min(512, SH - j * 512)
                xt = mmp.tile([IN, 512], f32, tag="xt", bufs=3)
                nc.sync.dma_start(out=xt[:, :n], in_=xT[:, j * 512:j * 512 + n])
                pu = psp.tile([HID, 512], f32, tag="pu", bufs=2)
                nc.tensor.matmul(pu[:, :n], lhsT=w1[:], rhs=xt[:, :n],
                                 start=True, stop=True)
                su = mmp.tile([HID, 512], f32, tag="su", bufs=3)
                nc.scalar.activation(su[:, :n], pu[:, :n], AF.Copy)
                for i in range(-(-n // 128)):
                    b = min(128, n - i * 128)
                    col = 4 * j + i
                    k = next(kk for kk in range(NW)
                             if BCOL[kk][0] <= col < BCOL[kk][1])
                    pt = psp.tile([128, HID], f32, tag="pt", bufs=2)
                    nc.tensor.transpose(out=pt[:b, :],
                                        in_=su[:, i * 128:i * 128 + b],
                                        identity=ident[:HID, :HID])
                    ut = mmp.tile([128, HID], f32, tag="ut", bufs=4)
                    nc.scalar.activation(ut[:b, :], pt[:b, :], AF.Copy,
                                         scale=dvc[:b, col:col + 1])
                    r0 = (col - BCOL[k][0]) * LANES
                    nc.sync.dma_start(out=tb[k].ap()[r0:r0 + b, :],
                                      in_=ut[:b, :])
                if j in ag_after:
                    ag_band(tb, t1, ag_after[j])

            # ---------------- two gather layers
            for layer, (tp, selfsrc) in enumerate([(t1, tb), (t2, hb)]):
                # level 1: per-call gather + segment reduce -> P
                cw = 0
                for (k, c0, cols, r0, runs) in calls:
                    gt = gap.tile([128, CALL_COLS, HID], f32, tag="gt", bufs=6)
                    gather(gt[:, :cols, :], band_in(tp, k),
                           ix1[:, cw:cw + cols * 8], cols * 128)
                    cw += cols * 8
                    # all runs go through DVE (L==1 is a converting copy):
                    # keeps gt readers vector-only so gather-buffer reuse
                    # never waits on the shared HW-DMA lane semaphores.
                    nstage = sum(nR for (L, nR) in runs)
                    stt = stp.tile([128, CALL_COLS, HID], f32,
                                   tag="stt", bufs=4, name="stt")
                    soff, coff = 0, 0
                    for (L, nR) in runs:
                        nc.vector.tensor_reduce(
                            out=stt[:, soff:soff + nR, :],
                            in_=gt[:, coff:coff + nR * L, :].rearrange(
                                "p (r l) f -> p r f l", l=L),
                            axis=mybir.AxisListType.X,
                            op=mybir.AluOpType.add)
                        soff += nR
                        coff += nR * L
                    nc.sync.dma_start(out=prowview(layer, k, r0, nstage),
                                      in_=stt[:, :nstage, :])

                # level 2: align partials + self, post-ops
                for ci, (kb, cc0, cc1) in enumerate(chunks):
                    ncc = cc1 - cc0
                    csl = slice(cc0, cc1)
                    parts = []
                    for k in range(NW):
                        g2 = l2p.tile([128, 13, HID], f32, tag="g2", bufs=6)
                        w0 = st["ix2off"][(ci, k)]
                        gather(g2[:, :ncc, :], P[layer][k].ap()[:, :],
                               ix2[:, w0:w0 + ncc * 8], ncc * 128)
                        parts.append(g2)
                    sf = l2p.tile([128, 13, HID], f32, tag="sf", bufs=3)
                    nc.sync.dma_start(out=sf[:, :ncc, :],
                                      in_=pm(selfsrc, kb, csl))
                    s = l2p.tile([128, 13, HID], f32, tag="s", bufs=2)
                    nc.vector.tensor_add(s[:, :ncc, :], parts[0][:, :ncc, :],
                                         parts[1][:, :ncc, :])
                    nc.vector.tensor_add(s[:, :ncc, :], s[:, :ncc, :],
                                         parts[2][:, :ncc, :])
                    nc.vector.tensor_add(s[:, :ncc, :], s[:, :ncc, :],
                                         parts[3][:, :ncc, :])
                    nc.vector.tensor_add(s[:, :ncc, :], s[:, :ncc, :],
                                         sf[:, :ncc, :])
                    sfl = s[:, :ncc, :].rearrange("p c f -> p (c f)")
                    if layer == 0:
                        a_t = l2p.tile([128, 13 * HID], f32, tag="ab", bufs=2)
                        nc.sync.dma_start(
                            out=a_t[:, :ncc * HID],
                            in_=Aexp[:, cc0 * HID:cc1 * HID])
                        b_t = l2p.tile([128, 13 * HID], f32, tag="bb", bufs=2)
                        nc.sync.dma_start(
                            out=b_t[:, :ncc * HID],
                            in_=Bexp[:, cc0 * HID:cc1 * HID])
                        nc.vector.tensor_mul(sfl, sfl, a_t[:, :ncc * HID])
                        nc.vector.tensor_add(sfl, sfl, b_t[:, :ncc * HID])
                        h = l2p.tile([128, 13 * HID], f32, tag="h", bufs=2)
                        nc.scalar.activation(h[:, :ncc * HID], sfl, AF.Relu)
                        nc.sync.dma_start(
                            out=pm(hb, kb, csl),
                            in_=h[:, :ncc * HID].rearrange(
                                "p (c f) -> p c f", f=HID))
                        if cc1 == BCOL[kb][1]:
                            ag_band(hb, t2, kb)
                    else:
                        d_t = l2p.tile([128, 13 * HID], f32, tag="ab", bufs=2)
                        nc.sync.dma_start(
                            out=d_t[:, :ncc * HID],
                            in_=Dexp[:, cc0 * HID:cc1 * HID])
                        nc.vector.tensor_mul(sfl, sfl, d_t[:, :ncc * HID])
                        # final: per 4-col block transpose + matmul + bias
                        nb = -(-ncc // 4)
                        for bI in range(nb):
                            bc = min(4, ncc - bI * 4)
                            pvt = psp.tile([HID, 512], f32, tag="pvt", bufs=2)
                            for i in range(bc):
                                nc.tensor.transpose(
                                    out=pvt[:, i * 128:(i + 1) * 128],
                                    in_=s[:, bI * 4 + i, :],
                                    identity=ident[:, :])
                            vt = mmp.tile([HID, 512], f32, tag="vt", bufs=3)
                            nc.scalar.activation(vt[:, :bc * 128],
                                                 pvt[:, :bc * 128], AF.Copy)
                            po = psp.tile([HID, 512], f32, tag="po", bufs=2)
                            nc.tensor.matmul(po[:, :bc * 128], lhsT=w2[:],
                                             rhs=vt[:, :bc * 128],
                                             start=True, stop=True)
                            ot = mmp.tile([HID, 512], f32, tag="ot", bufs=3)
                            nc.vector.tensor_scalar_add(ot[:, :bc * 128],
                                                        po[:, :bc * 128],
                                                        b2[:, 0:1])
                            c0g = (cc0 + bI * 4) * 128
                            nc.sync.dma_start(
                                out=outT[:, c0g:c0g + bc * 128],
                                in_=ot[:, :bc * 128])

    if do_compile:
        nc.compile()
    return nc


# ---------------------------------------------------------------- kernel
def kernel(x, edge_index, W1, b1, W_mu, b_mu, W_ls, b_ls):
    x = np.asarray(x, np.float32)
    edge_index = np.asarray(edge_index)
    st = build_structure(edge_index)
    in_maps = make_inputs(st, x, np.asarray(W1), np.asarray(b1),
                          np.asarray(W_mu), np.asarray(b_mu),
                          np.asarray(W_ls), np.asarray(b_ls))
    nc = build_program(st)
    from concourse import bass_utils
    trace = bool(int(os.environ.get("GCN_TRACE", "0")))
    if trace:
        try:
            import axon_profile_shim
            axon_profile_shim.install()
        except Exception:
            trace = False
    res = bass_utils.run_bass_kernel_spmd(
        nc, in_maps, core_ids=list(range(NC)), trace=trace)
    if trace:
        kernel.last_exec_time_ns = res.exec_time_ns
    mu = np.empty((N, OUT), np.float32)
    ls = np.empty((N, OUT), np.float32)
    for c in range(NC):
        o = res.results[c]["outT"]                     # [64, GRID]
        sel = o[:, :SH].T                              # [12500, 64]
        mu[c * SH:(c + 1) * SH] = sel[:, :OUT]
        ls[c * SH:(c + 1) * SH] = sel[:, OUT:]
    return mu, ls
